# revision 11
# baseline (speedup 1.0000x reference)
"""Grouped-Query Latent Attention on 8 Trainium2 NeuronCores.

Sharding: core c handles batch b = c // 4 and query-chunk j = c % 4
(512 queries each).  Each core computes the full latent / K / V for its
batch (replicated within the 4-core batch group), the Q projection for
its own 512 queries across all 16 heads, attention for those queries,
and the full output projection for its query chunk.  Output rows are
disjoint across cores, so the unshard step is a pure gather+transpose
(no cross-core reduction needed).

Everything on-chip works in a transposed layout (feature dim on SBUF
partitions, sequence on the free dim) so no on-chip transposes of the
activations are needed; the host feeds hidden_states pre-transposed and
transposes the output back.

Precision: projections (latent/Q/K/V and out-proj) run in fp32 on the
PE; the attention core (scores, exp, probs @ V) runs with bf16 operands
and fp32 PSUM accumulation.  Softmax skips the max-subtraction (scores
are bounded by ~±4 for this problem family), and the row-sum for the
softmax denominator rides along the probs @ V matmul as an extra ones
column appended to V.
"""

import sys

sys.path.insert(0, "/opt/trn_rl_repo")

import numpy as np

import concourse.bass as bass
import concourse.mybir as mybir
import concourse.tile as tile
from concourse import bacc
from concourse.bass_utils import run_bass_kernel_spmd
from concourse.masks import make_identity

P = 128
HIDDEN = 2048
S = 2048
B = 2
N_HEADS = 16
HEAD_DIM = 128
N_GROUPS = 4
LATENT = 512
KV = 512
N_CORES = 8
SQ = S * B // N_CORES  # 512 queries per core
SCALE = 1.0 / float(np.sqrt(HEAD_DIM))

F32 = mybir.dt.float32
BF16 = mybir.dt.bfloat16

H_CH = HIDDEN // P  # 16 hidden chunks
L_CH = LATENT // P  # 4 latent chunks
S_T = S // 512  # 4 sequence tiles (512 wide)
S_CH = S // P  # 16 sequence chunks (128 wide)
Q_SUB = SQ // P  # 4 query sub-tiles per core
HPG = N_HEADS // N_GROUPS  # 4 heads per kv group


def _emit(tc, aps):
    nc = tc.nc

    hs_t = aps["hs_t"]  # [HIDDEN, S]    hidden_states[b].T
    hs_q = aps["hs_q"]  # [HIDDEN, SQ]   this core's query columns
    wq = aps["wq"]  # [HIDDEN, HIDDEN]
    wl = aps["wl"]  # [HIDDEN, LATENT]
    wk = aps["wk"]  # [LATENT, KV]
    wv = aps["wv"]  # [LATENT, KV]
    wo = aps["wo"]  # [HIDDEN, HIDDEN]
    bcols = aps["bcols"]  # [P, 40]  packed [bq | bl | bk | bo]
    bv = aps["bv"]  # [P, KV]  (row-broadcast)
    out_p = aps["out_p"]  # [HIDDEN, SQ]

    hs_r = hs_t.rearrange("(hc p) s -> p hc s", p=P)  # [128, 16, S]
    hsq_r = hs_q.rearrange("(hc p) s -> p hc s", p=P)  # [128, 16, SQ]
    wq_r = wq.rearrange("(hc p) d -> p hc d", p=P)  # [128, 16, HIDDEN]
    wl_r = wl.rearrange("(hc p) l -> p hc l", p=P)  # [128, 16, LATENT]
    wk_r = wk.rearrange("(lc p) d -> p lc d", p=P)  # [128, 4, KV]
    wv_r = wv.rearrange("(lc p) d -> p lc d", p=P)  # [128, 4, KV]
    wo_r = wo.rearrange("(dc p) h -> p dc h", p=P)  # [128, 16, HIDDEN]
    out_r = out_p.rearrange("(ht p) s -> p ht s", p=P)  # [128, 16, SQ]

    with tc.tile_pool(name="persist", bufs=1) as persist:
        # packed small constants: [bq(16) | bl(4) | bk(4) | bo(16)]
        bcols_sb = persist.tile([P, 2 * H_CH + 2 * L_CH], F32)
        bv_sb = persist.tile([P, KV], F32)
        ident = persist.tile([P, P], F32)
        nc.sync.dma_start(bcols_sb, bcols)
        nc.sync.dma_start(bv_sb, bv)
        make_identity(nc, ident)
        bq_sb = bcols_sb[:, 0:H_CH]
        bl_sb = bcols_sb[:, H_CH : H_CH + L_CH]
        bk_sb = bcols_sb[:, H_CH + L_CH : H_CH + 2 * L_CH]
        bo_sb = bcols_sb[:, H_CH + 2 * L_CH : 2 * H_CH + 2 * L_CH]

        # activations that persist across phases.  latent.T (phases L..V)
        # and ctx.T (phases A..O) have disjoint lifetimes and identical
        # byte size -> share one slot via the same tag.
        latent_sb = persist.tile([P, L_CH, S], F32, tag="big")  # latent.T [l, s]
        q_bf = persist.tile([P, N_HEADS, SQ], BF16)  # q.T per head [d, sq]
        k_bf = persist.tile([P, N_GROUPS, S], BF16)  # k.T per group [d, s]
        v_aug = persist.tile([P, S_CH, N_GROUPS, HEAD_DIM + 1], BF16)

        # ---- Phase L: latent.T = Wl.T @ hs.T  (+ bl) --------------------
        with (
            tc.tile_pool(name="wl", bufs=1) as wlp,
            tc.tile_pool(name="hs", bufs=2) as hsp,
            tc.tile_pool(name="psl", bufs=8, space="PSUM") as psl,
        ):
            wl_sb = wlp.tile([P, H_CH, LATENT], F32)
            nc.sync.dma_start(wl_sb, wl_r)
            HH = H_CH // 2
            for st in range(S_T):
                # half-tiles (8 h-chunks each): half A is fully consumed
                # before half B is read, so 2 slots give true
                # double-buffering across s-tiles in 32KB/partition.
                pss = [
                    psl.tile([P, 512], F32, tag="ps", name=f"psl_{st}_{i}")
                    for i in range(L_CH)
                ]
                for hh in range(2):
                    ht_ = hsp.tile([P, HH, 512], F32, tag="hs")
                    nc.sync.dma_start(
                        ht_, hs_r[:, hh * HH : (hh + 1) * HH, st * 512 : (st + 1) * 512]
                    )
                    for lt in range(L_CH):
                        for hi in range(HH):
                            hc = hh * HH + hi
                            nc.tensor.matmul(
                                pss[lt],
                                wl_sb[:, hc, lt * P : (lt + 1) * P],
                                ht_[:, hi, :],
                                start=(hc == 0),
                                stop=(hc == H_CH - 1),
                            )
                for lt in range(L_CH):
                    nc.vector.tensor_scalar_add(
                        latent_sb[:, lt, st * 512 : (st + 1) * 512],
                        pss[lt],
                        bl_sb[:, lt : lt + 1],
                    )

        # ---- Phase Q: q.T = Wq.T @ hs_q  (+ bq), bf16 -------------------
        with (
            tc.tile_pool(name="hsq", bufs=1) as hsqp,
            tc.tile_pool(name="wq", bufs=3) as wqp,
            tc.tile_pool(name="psq", bufs=4, space="PSUM") as psq,
        ):
            hsq_sb = hsqp.tile([P, H_CH, SQ], F32)
            nc.sync.dma_start(hsq_sb, hsq_r)
            for dt in range(H_CH):
                wq_tile = wqp.tile([P, H_CH, P], F32, tag="wq")
                nc.sync.dma_start(wq_tile, wq_r[:, :, dt * P : (dt + 1) * P])
                ps = psq.tile([P, SQ], F32, tag="ps")
                for hc in range(H_CH):
                    nc.tensor.matmul(
                        ps,
                        wq_tile[:, hc, :],
                        hsq_sb[:, hc, :],
                        start=(hc == 0),
                        stop=(hc == H_CH - 1),
                    )
                nc.vector.tensor_scalar_add(q_bf[:, dt, :], ps, bq_sb[:, dt : dt + 1])

        # ---- Phase K/V ---------------------------------------------------
        with (
            tc.tile_pool(name="wkv", bufs=1) as wkvp,
            tc.tile_pool(name="pskv", bufs=8, space="PSUM") as pskv,
        ):
            wk_sb = wkvp.tile([P, L_CH, KV], F32)
            wv_sb = wkvp.tile([P, L_CH, KV], F32)
            nc.sync.dma_start(wk_sb, wk_r)
            nc.sync.dma_start(wv_sb, wv_r)
            # k.T per group: [d, s]
            for g in range(N_GROUPS):
                for st in range(S_T):
                    ps = pskv.tile([P, 512], F32, tag="ps")
                    for lc in range(L_CH):
                        nc.tensor.matmul(
                            ps,
                            wk_sb[:, lc, g * P : (g + 1) * P],
                            latent_sb[:, lc, st * 512 : (st + 1) * 512],
                            start=(lc == 0),
                            stop=(lc == L_CH - 1),
                        )
                    nc.vector.tensor_scalar_add(
                        k_bf[:, g, st * 512 : (st + 1) * 512],
                        ps,
                        bk_sb[:, g : g + 1],
                    )
            # v (untransposed): [s, dv] in 128-row chunks, all 4 groups wide
            for sc in range(S_CH):
                ps = pskv.tile([P, KV], F32, tag="ps")
                for lc in range(L_CH):
                    nc.tensor.matmul(
                        ps,
                        latent_sb[:, lc, sc * P : (sc + 1) * P],
                        wv_sb[:, lc, :],
                        start=(lc == 0),
                        stop=(lc == L_CH - 1),
                    )
                nc.vector.tensor_add(
                    v_aug[:, sc, :, 0:HEAD_DIM],
                    ps.rearrange("p (g d) -> p g d", g=N_GROUPS),
                    bv_sb.rearrange("p (g d) -> p g d", g=N_GROUPS),
                )
            nc.vector.memset(v_aug[:, :, :, HEAD_DIM : HEAD_DIM + 1], 1.0)

        # ---- Phase A: attention -----------------------------------------
        # reuses the latent.T slot (lifetimes are disjoint)
        ctxT_sb = persist.tile([P, N_HEADS, SQ], F32, tag="big")
        with (
            tc.tile_pool(name="exps", bufs=2) as expp,
            tc.tile_pool(name="ctmp", bufs=4) as ctmpp,
            tc.tile_pool(name="rec", bufs=4) as recp,
            tc.tile_pool(name="pss", bufs=2, space="PSUM") as pss,
            tc.tile_pool(name="psc", bufs=2, space="PSUM") as psc,
            tc.tile_pool(name="pst", bufs=2, space="PSUM") as pst,
        ):
            for h in range(N_HEADS):
                g = h // HPG
                exp_sb = expp.tile([P, S_CH, SQ], BF16, tag="exp")
                # scores.T chunks [sk, sq] -> exp -> bf16
                for half in range(S_CH // 2):
                    ps_s = pss.tile([P, 2, SQ], F32, tag="ps")
                    for c2 in range(2):
                        c = half * 2 + c2
                        nc.tensor.matmul(
                            ps_s[:, c2, :],
                            k_bf[:, g, c * P : (c + 1) * P],
                            q_bf[:, h, :],
                            start=True,
                            stop=True,
                        )
                    nc.scalar.activation(
                        exp_sb[:, half * 2 : half * 2 + 2, :],
                        ps_s,
                        mybir.ActivationFunctionType.Exp,
                        scale=SCALE,
                    )
                # probs @ V with ones column for the softmax denominator
                for j in range(Q_SUB):
                    ps_c = psc.tile([P, HEAD_DIM + 1], F32, tag="ps")
                    for c in range(S_CH):
                        nc.tensor.matmul(
                            ps_c,
                            exp_sb[:, c, j * P : (j + 1) * P],
                            v_aug[:, c, g, :],
                            start=(c == 0),
                            stop=(c == S_CH - 1),
                        )
                    rec = recp.tile([P, 1], F32, tag="rec")
                    nc.vector.reciprocal(rec, ps_c[:, HEAD_DIM : HEAD_DIM + 1])
                    ctx_tmp = ctmpp.tile([P, P], F32, tag="ctx")
                    nc.vector.tensor_scalar_mul(ctx_tmp, ps_c[:, 0:HEAD_DIM], rec)
                    ps_t = pst.tile([P, P], F32, tag="ps")
                    nc.tensor.transpose(ps_t, ctx_tmp, ident)
                    nc.vector.tensor_copy(
                        ctxT_sb[:, h, j * P : (j + 1) * P], ps_t
                    )

        # ---- Phase O: out.T = Wo.T @ ctx.T (+ bo) -----------------------
        with (
            tc.tile_pool(name="wo", bufs=3) as wop,
            tc.tile_pool(name="ob", bufs=4) as obp,
            tc.tile_pool(name="pso", bufs=4, space="PSUM") as pso,
        ):
            for ht in range(H_CH):
                wo_tile = wop.tile([P, H_CH, P], F32, tag="wo")
                nc.sync.dma_start(wo_tile, wo_r[:, :, ht * P : (ht + 1) * P])
                ps = pso.tile([P, SQ], F32, tag="ps")
                for dc in range(H_CH):
                    nc.tensor.matmul(
                        ps,
                        wo_tile[:, dc, :],
                        ctxT_sb[:, dc, :],
                        start=(dc == 0),
                        stop=(dc == H_CH - 1),
                    )
                ob = obp.tile([P, SQ], F32, tag="ob")
                nc.vector.tensor_scalar_add(ob, ps, bo_sb[:, ht : ht + 1])
                nc.sync.dma_start(out_r[:, ht, :], ob)


def build(debug=False):
    nc = bacc.Bacc("TRN2", target_bir_lowering=False, debug=debug)
    aps = {}
    for name, shape in [
        ("hs_t", [HIDDEN, S]),
        ("hs_q", [HIDDEN, SQ]),
        ("wq", [HIDDEN, HIDDEN]),
        ("wl", [HIDDEN, LATENT]),
        ("wk", [LATENT, KV]),
        ("wv", [LATENT, KV]),
        ("wo", [HIDDEN, HIDDEN]),
        ("bcols", [P, 2 * H_CH + 2 * L_CH]),
        ("bv", [P, KV]),
    ]:
        aps[name] = nc.dram_tensor(name, shape, F32, kind="ExternalInput").ap()
    aps["out_p"] = nc.dram_tensor("out_p", [HIDDEN, SQ], F32, kind="ExternalOutput").ap()
    with tile.TileContext(nc) as tc:
        _emit(tc, aps)
    nc.compile()
    return nc


def make_in_maps(inputs):
    hs = np.ascontiguousarray(np.asarray(inputs["hidden_states"], dtype=np.float32))
    shared = {
        "wq": np.ascontiguousarray(np.asarray(inputs["Wq"], np.float32)),
        "wl": np.ascontiguousarray(np.asarray(inputs["Wl"], np.float32)),
        "wk": np.ascontiguousarray(np.asarray(inputs["Wk"], np.float32)),
        "wv": np.ascontiguousarray(np.asarray(inputs["Wv"], np.float32)),
        "wo": np.ascontiguousarray(np.asarray(inputs["Wo"], np.float32)),
        "bcols": np.ascontiguousarray(
            np.concatenate(
                [
                    np.asarray(inputs["bq"], np.float32).reshape(H_CH, P).T,
                    np.asarray(inputs["bl"], np.float32).reshape(L_CH, P).T,
                    np.asarray(inputs["bk"], np.float32).reshape(L_CH, P).T,
                    np.asarray(inputs["bo"], np.float32).reshape(H_CH, P).T,
                ],
                axis=1,
            )
        ),
        "bv": np.ascontiguousarray(
            np.tile(np.asarray(inputs["bv"], np.float32)[None, :], (P, 1))
        ),
    }
    hs_t = [np.ascontiguousarray(hs[b].T) for b in range(B)]
    in_maps = []
    for c in range(N_CORES):
        b, j = divmod(c, N_CORES // B)
        m = dict(shared)
        m["hs_t"] = hs_t[b]
        m["hs_q"] = np.ascontiguousarray(hs_t[b][:, j * SQ : (j + 1) * SQ])
        in_maps.append(m)
    return in_maps


def unshard(parts):
    """parts: list of 8 arrays [HIDDEN, SQ] -> [B, S, HIDDEN]."""
    out = np.empty((B, S, HIDDEN), np.float32)
    jpb = N_CORES // B
    for b in range(B):
        out_t = np.concatenate([parts[b * jpb + j] for j in range(jpb)], axis=1)
        out[b] = out_t.T
    return out


_NC_CACHE = None


def kernel(**inputs):
    global _NC_CACHE
    if _NC_CACHE is None:
        _NC_CACHE = build(debug=False)
    nc = _NC_CACHE
    in_maps = make_in_maps(inputs)
    res = run_bass_kernel_spmd(nc, in_maps, core_ids=list(range(N_CORES)))
    parts = [res.results[c]["out_p"] for c in range(N_CORES)]
    return unshard(parts)


if __name__ == "__main__":
    import reference as R

    inputs = R.setup_inputs()
    out = kernel(**inputs)
    exp = np.asarray(R.reference(**inputs))
    err = np.abs(out - exp).max() / np.abs(exp).max()
    print("rel err:", err)


# revision 12
# speedup vs baseline: 2.0984x; 2.0984x over previous
"""Grouped-Query Latent Attention on 8 Trainium2 NeuronCores.

Sharding: core c handles batch b = c // 4 and query-chunk j = c % 4
(512 queries each).  Each core computes the full latent / K / V for its
batch (replicated within the 4-core batch group), the Q projection for
its own 512 queries across all 16 heads, attention for those queries,
and the full output projection for its query chunk.  Output rows are
disjoint across cores, so the unshard step is a pure gather+transpose
(no cross-core reduction needed).

Everything on-chip works in a transposed layout (feature dim on SBUF
partitions, sequence on the free dim) so no on-chip transposes of the
activations are needed; the host feeds hidden_states pre-transposed and
transposes the output back.

Precision: projections and scores run the PE in float32r (TF32-like,
full PE rate for free-dim >= 256; measured ~1.5e-4 matmul rel-err vs
~2.4e-3 for bf16) with fp32 PSUM accumulation.  The probs @ V matmul
(free dim 129 < 256, where f32r drops to 1/4 rate) uses bf16 operands.
Softmax skips max-subtraction (scores bounded ~|3| here), and the
softmax denominator rides along probs @ V as a ones column appended
to V.
"""

import sys

sys.path.insert(0, "/opt/trn_rl_repo")

import numpy as np

import concourse.bass as bass
import concourse.mybir as mybir
import concourse.tile as tile
from concourse import bacc
from concourse.bass_utils import run_bass_kernel_spmd
from concourse.masks import make_identity

P = 128
HIDDEN = 2048
S = 2048
B = 2
N_HEADS = 16
HEAD_DIM = 128
N_GROUPS = 4
LATENT = 512
KV = 512
N_CORES = 8
SQ = S * B // N_CORES  # 512 queries per core
SCALE = 1.0 / float(np.sqrt(HEAD_DIM))

F32 = mybir.dt.float32
F32R = mybir.dt.float32r
BF16 = mybir.dt.bfloat16

H_CH = HIDDEN // P  # 16 hidden chunks
L_CH = LATENT // P  # 4 latent chunks
S_T = S // 512  # 4 sequence tiles (512 wide)
S_CH = S // P  # 16 sequence chunks (128 wide)
Q_SUB = SQ // P  # 4 query sub-tiles per core
HPG = N_HEADS // N_GROUPS  # 4 heads per kv group


def _emit(tc, aps):
    nc = tc.nc

    hs_t = aps["hs_t"]  # [HIDDEN, S]   hidden_states[b].T   (f32r)
    hs_q = aps["hs_q"]  # [HIDDEN, SQ]  this core's query columns (f32r)
    wq = aps["wq"]  # [HIDDEN, HIDDEN]  (f32r)
    wl = aps["wl"]  # [HIDDEN, LATENT]  (f32r)
    wk = aps["wk"]  # [LATENT, KV]      (f32r)
    wv = aps["wv"]  # [LATENT, KV]      (f32r)
    wo = aps["wo"]  # [HIDDEN, HIDDEN]  (f32r)
    bcols = aps["bcols"]  # [P, 40]  packed [bq | bl | bk | bo]
    bv = aps["bv"]  # [P, KV]  (row-broadcast)
    out_p = aps["out_p"]  # [HIDDEN, SQ]

    hs_r = hs_t.rearrange("(hc p) s -> p hc s", p=P)  # [128, 16, S]
    hsq_r = hs_q.rearrange("(hc p) s -> p hc s", p=P)  # [128, 16, SQ]
    wq_r = wq.rearrange("(hc p) d -> p hc d", p=P)  # [128, 16, HIDDEN]
    wl_r = wl.rearrange("(hc p) l -> p hc l", p=P)  # [128, 16, LATENT]
    wk_r = wk.rearrange("(lc p) d -> p lc d", p=P)  # [128, 4, KV]
    wv_r = wv.rearrange("(lc p) d -> p lc d", p=P)  # [128, 4, KV]
    wo_r = wo.rearrange("(dc p) h -> p dc h", p=P)  # [128, 16, HIDDEN]
    out_r = out_p.rearrange("(ht p) s -> p ht s", p=P)  # [128, 16, SQ]

    with tc.tile_pool(name="persist", bufs=1) as persist:
        # packed small constants: [bq(16) | bl(4) | bk(4) | bo(16)]
        bcols_sb = persist.tile([P, 2 * H_CH + 2 * L_CH], F32)
        bv_sb = persist.tile([P, KV], F32)
        ident = persist.tile([P, P], F32)
        nc.sync.dma_start(bcols_sb, bcols)
        nc.sync.dma_start(bv_sb, bv)
        make_identity(nc, ident)
        bq_sb = bcols_sb[:, 0:H_CH]
        bl_sb = bcols_sb[:, H_CH : H_CH + L_CH]
        bk_sb = bcols_sb[:, H_CH + L_CH : H_CH + 2 * L_CH]
        bo_sb = bcols_sb[:, H_CH + 2 * L_CH : 2 * H_CH + 2 * L_CH]

        # activations that persist across phases.  latent.T (phases L..V)
        # and ctx.T (phases A..O) have disjoint lifetimes and identical
        # byte size -> share one slot via the same tag.
        q_r = persist.tile([P, N_HEADS, SQ], F32R)  # q.T per head [d, sq]
        k_r = persist.tile([P, N_GROUPS, S], F32R)  # k.T per group [d, s]
        v_aug = persist.tile([P, S_CH, N_GROUPS, HEAD_DIM + 1], BF16)

        # ---- Phase Q: q.T = Wq.T @ hs_q  (+ bq) -------------------------
        with (
            tc.tile_pool(name="hsq", bufs=1) as hsqp,
            tc.tile_pool(name="wq", bufs=3) as wqp,
            tc.tile_pool(name="psq", bufs=4, space="PSUM") as psq,
        ):
            hsq_sb = hsqp.tile([P, H_CH, SQ], F32R)
            nc.sync.dma_start(hsq_sb, hsq_r)
            for dt in range(H_CH):
                wq_tile = wqp.tile([P, H_CH, P], F32R, tag="wq")
                nc.sync.dma_start(wq_tile, wq_r[:, :, dt * P : (dt + 1) * P])
                ps = psq.tile([P, SQ], F32, tag="ps")
                for hc in range(H_CH):
                    nc.tensor.matmul(
                        ps,
                        wq_tile[:, hc, :],
                        hsq_sb[:, hc, :],
                        start=(hc == 0),
                        stop=(hc == H_CH - 1),
                    )
                nc.vector.tensor_scalar_add(q_r[:, dt, :], ps, bq_sb[:, dt : dt + 1])

        # ---- Phase L: latent.T = Wl.T @ hs.T  (+ bl) --------------------
        latent_sb = persist.tile([P, L_CH, S], F32R, tag="big")
        with (
            tc.tile_pool(name="wlr", bufs=1) as wlrp,
            tc.tile_pool(name="hs", bufs=2) as hsp,
            tc.tile_pool(name="psl", bufs=8, space="PSUM") as psl,
        ):
            wl_sb = wlrp.tile([P, H_CH, LATENT], F32R)
            nc.sync.dma_start(wl_sb, wl_r)
            HH = H_CH // 2
            for st in range(S_T):
                # half-tiles (8 h-chunks each): half A is fully consumed
                # before half B is read, so 2 slots double-buffer across
                # s-tiles in 32KB/partition.
                pss = [
                    psl.tile([P, 512], F32, tag="ps", name=f"psl_{st}_{i}")
                    for i in range(L_CH)
                ]
                for hh in range(2):
                    ht_ = hsp.tile([P, HH, 512], F32R, tag="hs")
                    nc.sync.dma_start(
                        ht_, hs_r[:, hh * HH : (hh + 1) * HH, st * 512 : (st + 1) * 512]
                    )
                    for lt in range(L_CH):
                        for hi in range(HH):
                            hc = hh * HH + hi
                            nc.tensor.matmul(
                                pss[lt],
                                wl_sb[:, hc, lt * P : (lt + 1) * P],
                                ht_[:, hi, :],
                                start=(hc == 0),
                                stop=(hc == H_CH - 1),
                            )
                for lt in range(L_CH):
                    nc.vector.tensor_scalar_add(
                        latent_sb[:, lt, st * 512 : (st + 1) * 512],
                        pss[lt],
                        bl_sb[:, lt : lt + 1],
                    )

        # ---- Phase K/V ---------------------------------------------------
        with (
            tc.tile_pool(name="wkv", bufs=1) as wkvp,
            tc.tile_pool(name="pskv", bufs=8, space="PSUM") as pskv,
        ):
            wk_sb = wkvp.tile([P, L_CH, KV], F32R)
            wv_sb = wkvp.tile([P, L_CH, KV], F32R)
            nc.sync.dma_start(wk_sb, wk_r)
            nc.sync.dma_start(wv_sb, wv_r)
            # k.T per group: [d, s]
            for g in range(N_GROUPS):
                for st in range(S_T):
                    ps = pskv.tile([P, 512], F32, tag="ps")
                    for lc in range(L_CH):
                        nc.tensor.matmul(
                            ps,
                            wk_sb[:, lc, g * P : (g + 1) * P],
                            latent_sb[:, lc, st * 512 : (st + 1) * 512],
                            start=(lc == 0),
                            stop=(lc == L_CH - 1),
                        )
                    nc.vector.tensor_scalar_add(
                        k_r[:, g, st * 512 : (st + 1) * 512],
                        ps,
                        bk_sb[:, g : g + 1],
                    )
            # v (untransposed): [s, dv] in 128-row chunks, all 4 groups wide
            for sc in range(S_CH):
                ps = pskv.tile([P, KV], F32, tag="ps")
                for lc in range(L_CH):
                    nc.tensor.matmul(
                        ps,
                        latent_sb[:, lc, sc * P : (sc + 1) * P],
                        wv_sb[:, lc, :],
                        start=(lc == 0),
                        stop=(lc == L_CH - 1),
                    )
                nc.vector.tensor_add(
                    v_aug[:, sc, :, 0:HEAD_DIM],
                    ps.rearrange("p (g d) -> p g d", g=N_GROUPS),
                    bv_sb.rearrange("p (g d) -> p g d", g=N_GROUPS),
                )
            nc.vector.memset(v_aug[:, :, :, HEAD_DIM : HEAD_DIM + 1], 1.0)

        # ---- Phase A: attention -----------------------------------------
        # reuses the latent.T slot (lifetimes are disjoint)
        ctxT_sb = persist.tile([P, N_HEADS, SQ], F32R, tag="big")
        with (
            tc.tile_pool(name="exps", bufs=2) as expp,
            tc.tile_pool(name="ctmp", bufs=4) as ctmpp,
            tc.tile_pool(name="rec", bufs=4) as recp,
            tc.tile_pool(name="pss", bufs=2, space="PSUM") as pssp,
            tc.tile_pool(name="psc", bufs=2, space="PSUM") as pscp,
            tc.tile_pool(name="pst", bufs=2, space="PSUM") as pstp,
        ):
            for h in range(N_HEADS):
                g = h // HPG
                exp_sb = expp.tile([P, S_CH, SQ], BF16, tag="exp")
                # scores.T chunks [sk, sq] -> exp -> bf16
                for half in range(S_CH // 2):
                    ps_s = pssp.tile([P, 2, SQ], F32, tag="ps")
                    for c2 in range(2):
                        c = half * 2 + c2
                        nc.tensor.matmul(
                            ps_s[:, c2, :],
                            k_r[:, g, c * P : (c + 1) * P],
                            q_r[:, h, :],
                            start=True,
                            stop=True,
                        )
                    nc.scalar.activation(
                        exp_sb[:, half * 2 : half * 2 + 2, :],
                        ps_s,
                        mybir.ActivationFunctionType.Exp,
                        scale=SCALE,
                    )
                # probs @ V with ones column for the softmax denominator
                for j in range(Q_SUB):
                    ps_c = pscp.tile([P, HEAD_DIM + 1], F32, tag="ps")
                    for c in range(S_CH):
                        nc.tensor.matmul(
                            ps_c,
                            exp_sb[:, c, j * P : (j + 1) * P],
                            v_aug[:, c, g, :],
                            start=(c == 0),
                            stop=(c == S_CH - 1),
                        )
                    rec = recp.tile([P, 1], F32, tag="rec")
                    nc.vector.reciprocal(rec, ps_c[:, HEAD_DIM : HEAD_DIM + 1])
                    ctx_tmp = ctmpp.tile([P, P], F32, tag="ctx")
                    nc.vector.tensor_scalar_mul(ctx_tmp, ps_c[:, 0:HEAD_DIM], rec)
                    ps_t = pstp.tile([P, P], F32, tag="ps")
                    nc.tensor.transpose(ps_t, ctx_tmp, ident)
                    nc.vector.tensor_copy(ctxT_sb[:, h, j * P : (j + 1) * P], ps_t)

        # ---- Phase O: out.T = Wo.T @ ctx.T (+ bo) -----------------------
        with (
            tc.tile_pool(name="wo", bufs=3) as wop,
            tc.tile_pool(name="ob", bufs=4) as obp,
            tc.tile_pool(name="pso", bufs=4, space="PSUM") as pso,
        ):
            for ht in range(H_CH):
                wo_tile = wop.tile([P, H_CH, P], F32R, tag="wo")
                nc.sync.dma_start(wo_tile, wo_r[:, :, ht * P : (ht + 1) * P])
                ps = pso.tile([P, SQ], F32, tag="ps")
                for dc in range(H_CH):
                    nc.tensor.matmul(
                        ps,
                        wo_tile[:, dc, :],
                        ctxT_sb[:, dc, :],
                        start=(dc == 0),
                        stop=(dc == H_CH - 1),
                    )
                ob = obp.tile([P, SQ], F32, tag="ob")
                nc.vector.tensor_scalar_add(ob, ps, bo_sb[:, ht : ht + 1])
                nc.sync.dma_start(out_r[:, ht, :], ob)


def build(debug=False):
    nc = bacc.Bacc("TRN2", target_bir_lowering=False, debug=debug)
    aps = {}
    for name, shape, dt_ in [
        ("hs_t", [HIDDEN, S], F32R),
        ("hs_q", [HIDDEN, SQ], F32R),
        ("wq", [HIDDEN, HIDDEN], F32R),
        ("wl", [HIDDEN, LATENT], F32R),
        ("wk", [LATENT, KV], F32R),
        ("wv", [LATENT, KV], F32R),
        ("wo", [HIDDEN, HIDDEN], F32R),
        ("bcols", [P, 2 * H_CH + 2 * L_CH], F32),
        ("bv", [P, KV], F32),
    ]:
        aps[name] = nc.dram_tensor(name, shape, dt_, kind="ExternalInput").ap()
    aps["out_p"] = nc.dram_tensor(
        "out_p", [HIDDEN, SQ], F32, kind="ExternalOutput"
    ).ap()
    with tile.TileContext(nc) as tc:
        _emit(tc, aps)
    nc.compile()
    return nc


def make_in_maps(inputs):
    hs = np.ascontiguousarray(np.asarray(inputs["hidden_states"], dtype=np.float32))
    shared = {
        "wq": np.ascontiguousarray(np.asarray(inputs["Wq"], np.float32)),
        "wl": np.ascontiguousarray(np.asarray(inputs["Wl"], np.float32)),
        "wk": np.ascontiguousarray(np.asarray(inputs["Wk"], np.float32)),
        "wv": np.ascontiguousarray(np.asarray(inputs["Wv"], np.float32)),
        "wo": np.ascontiguousarray(np.asarray(inputs["Wo"], np.float32)),
        "bcols": np.ascontiguousarray(
            np.concatenate(
                [
                    np.asarray(inputs["bq"], np.float32).reshape(H_CH, P).T,
                    np.asarray(inputs["bl"], np.float32).reshape(L_CH, P).T,
                    np.asarray(inputs["bk"], np.float32).reshape(L_CH, P).T,
                    np.asarray(inputs["bo"], np.float32).reshape(H_CH, P).T,
                ],
                axis=1,
            )
        ),
        "bv": np.ascontiguousarray(
            np.tile(np.asarray(inputs["bv"], np.float32)[None, :], (P, 1))
        ),
    }
    hs_t = [np.ascontiguousarray(hs[b].T) for b in range(B)]
    in_maps = []
    for c in range(N_CORES):
        b, j = divmod(c, N_CORES // B)
        m = dict(shared)
        m["hs_t"] = hs_t[b]
        m["hs_q"] = np.ascontiguousarray(hs_t[b][:, j * SQ : (j + 1) * SQ])
        in_maps.append(m)
    return in_maps


def unshard(parts):
    """parts: list of 8 arrays [HIDDEN, SQ] -> [B, S, HIDDEN]."""
    out = np.empty((B, S, HIDDEN), np.float32)
    jpb = N_CORES // B
    for b in range(B):
        out_t = np.concatenate([parts[b * jpb + j] for j in range(jpb)], axis=1)
        out[b] = out_t.T
    return out


_NC_CACHE = None


def kernel(**inputs):
    global _NC_CACHE
    if _NC_CACHE is None:
        _NC_CACHE = build(debug=False)
    nc = _NC_CACHE
    in_maps = make_in_maps(inputs)
    res = run_bass_kernel_spmd(nc, in_maps, core_ids=list(range(N_CORES)))
    parts = [res.results[c]["out_p"] for c in range(N_CORES)]
    return unshard(parts)


if __name__ == "__main__":
    import reference as R

    inputs = R.setup_inputs()
    out = kernel(**inputs)
    exp = np.asarray(R.reference(**inputs))
    err = np.abs(out - exp).max() / np.abs(exp).max()
    print("rel err:", err)
